# revision 1
# baseline (speedup 1.0000x reference)
"""GAT (gnn_message_passing) Trainium2 Bass kernel — 8-core SPMD.

Contract: kernel(**inputs) -> np.ndarray with FULL inputs / FULL output.
Self-contained: hardcodes shapes; only imports the container's concourse stack.
"""
import sys

for _p in ("/opt/trn_rl_repo", "/root/.axon_site/_ro/trn_rl_repo"):
    if _p not in sys.path:
        sys.path.append(_p)

import numpy as np
import os
_PHASE = int(os.environ.get("KPHASE", "4"))
_KSUB = int(os.environ.get("KSUB", "9"))
_BF16 = os.environ.get("KBF16", "0") == "1"

# ---------------- problem constants (hardcoded per contract) ----------------
N = 50000
NF = 513
NFP = 640            # padded feature dim (5 * 128)
NMEL = 128
H, C = 4, 32
HC = H * C           # 128
E = 800000
NEG_ATT = 0.2
NEG_MLP = 0.01

NCORES = 8
TPC = 49             # tiles per core
NT = 128             # nodes per tile
NPC = TPC * NT       # 6272 nodes per core
NPAD = NCORES * NPC  # 50176
RD = 192             # Hext row elems (768 B)
SPLIT = 25000        # src half split
BASE_B = N + (NPAD - N) - 32768 + 2768  # see below; recomputed in prep
W_IND = 128          # indicator window (full tile width for v1)

_CACHE = {}


def _prep(edge_index):
    """Host-side edge preprocessing. Returns per-core index/metadata arrays."""
    src = np.asarray(edge_index[0], dtype=np.int64)
    dst = np.asarray(edge_index[1], dtype=np.int64)
    loop = np.arange(N, dtype=np.int64)
    src = np.concatenate([src, loop])
    dst = np.concatenate([dst, loop])

    tile_g = dst // NT                # global tile id 0..390
    # quarter-major permuted table: quarter q of every core is AllGathered as
    # one region; region q = rows [RQ[q], RQ[q] + 8*qsz[q]) rank-major.
    QSR = np.array([0, 3072, NPC], dtype=np.int64)  # local row starts per region
    QSZ = np.diff(QSR)
    RQ = np.array([0, 8 * 3072], dtype=np.int64)
    s_k = src // NPC
    s_l = src % NPC
    s_q = (s_l >= 3072).astype(np.int64)
    src_row = RQ[s_q] + s_k * QSZ[s_q] + (s_l - QSR[s_q])
    half = s_q                             # table region pair
    # order: (tile, pair, dst, src)
    order = np.lexsort((src, dst, half, tile_g))
    src_row = src_row[order]
    src, dst, tile_g, half = src[order], dst[order], tile_g[order], half[order]

    NTILES_G = NPAD // NT            # 392
    # counts per (tile, half)
    cnt = np.zeros((NTILES_G, 2), dtype=np.int64)
    np.add.at(cnt, (tile_g, half), 1)
    starts = np.zeros((NTILES_G, 2), dtype=np.int64)
    flat = cnt.reshape(-1)
    starts.reshape(-1)[1:] = np.cumsum(flat)[:-1]

    # chunks per (slot, half): max over cores
    cores = np.arange(NCORES)
    cpt = np.zeros((TPC, 2), dtype=np.int64)
    for s in range(TPC):
        t_ids = cores * TPC + s
        for hf in range(2):
            cpt[s, hf] = max(1, int(np.ceil(cnt[t_ids, hf].max() / NT)))
    TOTC_S = cpt.sum(axis=1)          # chunks per slot
    TOTC = int(TOTC_S.sum())          # chunks per core (same structurally)
    TOTIDX = TOTC * NT

    base_b = 24576                    # pair-1 region start row

    # per-core outputs
    src_rel = np.zeros((NCORES, TOTC, NT), dtype=np.int64)
    ad_idx = np.zeros((NCORES, TOTC, NT), dtype=np.int64)
    dst_rel = np.full((NCORES, TOTC, NT), 999.0, dtype=np.float32)
    dloc_all = np.zeros((NCORES, TOTC, NT), dtype=np.int64)
    valid = np.zeros((NCORES, TOTC, NT), dtype=bool)

    for k in range(NCORES):
        coff = 0
        for s in range(TPC):
            t = k * TPC + s
            for hf in range(2):
                nch = int(cpt[s, hf])
                st, cn = starts[t, hf], int(cnt[t, hf])
                sl_src = src_row[st:st + cn]
                sl_dst = dst[st:st + cn]
                base = 0 if hf == 0 else base_b
                src_rel[k, coff:coff + nch].reshape(-1)[:cn] = sl_src - base
                ad_idx[k, coff:coff + nch].reshape(-1)[:cn] = sl_dst % NPC
                dloc_all[k, coff:coff + nch].reshape(-1)[:cn] = sl_dst % NT
                valid[k, coff:coff + nch].reshape(-1)[:cn] = True
                coff += nch
        assert coff == TOTC

    assert src_rel.min() >= 0 and src_rel.max() <= 32767

    # window offsets per chunk (uniform across cores): 64-wide at 32-aligned
    # offsets {0,32,64} when the cross-core dst span fits, else full 128.
    woff = np.zeros(TOTC, dtype=np.int64)
    wlen = np.full(TOTC, 128, dtype=np.int64)
    for c in range(TOTC):
        v = valid[:, c, :]
        if v.any():
            dl = dloc_all[:, c, :][v]
            lo, hi = int(dl.min()), int(dl.max())
            wo = 0 if lo < 64 else 64
            if hi < wo + 64:
                woff[c] = wo
                wlen[c] = 64

    for k in range(NCORES):
        dr = dloc_all[k] - woff[:, None]
        dst_rel[k][valid[k]] = dr[valid[k]].astype(np.float32)

    # wrapped int16 index layout: [128, TOTIDX//16]; idx i of a call at
    # partition i%16 (replicated x8), col i//16. Calls slice columns.
    def wrap(a):  # a: [NCORES, TOTC, NT] -> [NCORES, 128, TOTIDX//16]
        fl = a.reshape(NCORES, TOTIDX)
        w = fl.reshape(NCORES, TOTIDX // 16, 16).transpose(0, 2, 1)  # [NC,16,cols]
        return np.tile(w, (1, 8, 1)).astype(np.int16)

    src_w = wrap(src_rel)
    ad_w = wrap(ad_idx)
    # dst_rel for SBUF [128, TOTC]: partition=edge pos, col=chunk
    dst_col = dst_rel.transpose(0, 2, 1).copy()  # [NCORES, 128, TOTC]

    meta = {
        "cpt": cpt, "woff": woff, "wlen": wlen, "TOTC": TOTC,
        "TOTIDX": TOTIDX, "base_b": base_b,
    }
    return src_w, ad_w, dst_col, meta


def _build(meta):
    import concourse.bass as bass
    import concourse.bacc as bacc
    import concourse.mybir as mybir
    import concourse.tile as tile

    f32 = mybir.dt.float32
    bf16 = mybir.dt.bfloat16
    i16 = mybir.dt.int16
    hdt = bf16 if _BF16 else f32
    RDX = 256 if _BF16 else RD
    ADR = 128 if _BF16 else 64
    AF = mybir.ActivationFunctionType
    OP = mybir.AluOpType

    cpt, woff, TOTC, TOTIDX = meta["cpt"], meta["woff"], meta["TOTC"], meta["TOTIDX"]
    wlen = meta["wlen"]
    base_b = meta["base_b"]

    nc = bacc.Bacc("TRN2", target_bir_lowering=False, debug=False)

    # ---- I/O ----
    x_sl = nc.dram_tensor("x_sl", [NPC, NFP], f32, kind="ExternalInput")
    idx_src = nc.dram_tensor("idx_src", [128, TOTIDX // 16], i16, kind="ExternalInput")
    idx_ad = nc.dram_tensor("idx_ad", [128, TOTIDX // 16], i16, kind="ExternalInput")
    dst_col = nc.dram_tensor("dst_col", [128, TOTC], f32, kind="ExternalInput")
    fb_p = nc.dram_tensor("fb_p", [NFP, NMEL], f32, kind="ExternalInput")
    Wg_d = nc.dram_tensor("Wg", [NMEL, HC], f32, kind="ExternalInput")
    attb_s = nc.dram_tensor("attb_s", [HC, 4], f32, kind="ExternalInput")
    attb_d = nc.dram_tensor("attb_d", [HC, 4], f32, kind="ExternalInput")
    bias_bc = nc.dram_tensor("bias_bc", [128, HC], f32, kind="ExternalInput")
    W1_d = nc.dram_tensor("W1", [HC, 256], f32, kind="ExternalInput")
    b1_d = nc.dram_tensor("b1", [128, 2], f32, kind="ExternalInput")
    W2_d = nc.dram_tensor("W2", [256, HC], f32, kind="ExternalInput")
    b2_d = nc.dram_tensor("b2", [128, 1], f32, kind="ExternalInput")
    W3_d = nc.dram_tensor("W3", [HC, 10], f32, kind="ExternalInput")
    b3_d = nc.dram_tensor("b3", [128, 1], f32, kind="ExternalInput")
    eye_d = nc.dram_tensor("eye", [128, 128], f32, kind="ExternalInput")
    iota_d = nc.dram_tensor("iota", [128, 128], f32, kind="ExternalInput")
    ones_d = nc.dram_tensor("ones", [128, 16], f32, kind="ExternalInput")
    outT = nc.dram_tensor("outT", [10, NPC], f32, kind="ExternalOutput")

    core_ids = list(range(NCORES))

    with tile.TileContext(nc) as tc:
        with (
            tc.tile_pool(name="dram", bufs=1, space="DRAM") as dpool,
            tc.tile_pool(name="const", bufs=1) as cpool,
        ):
            Hext_loc = dpool.tile([NPC, RDX], hdt)
            Hfull_a = dpool.tile([8 * 3072, RDX], hdt, addr_space="Shared")
            Hfull_b = dpool.tile([8 * 3200, RDX], hdt, addr_space="Shared")
            adrep = dpool.tile([NPC, ADR], hdt)

            # ---- constants to SBUF ----
            fb_t = cpool.tile([128, 5, NMEL], f32)
            nc.sync.dma_start(fb_t[:], fb_p.rearrange("(b p) m -> p b m", p=128))
            Wg_t = cpool.tile([128, HC], f32)
            nc.sync.dma_start(Wg_t[:], Wg_d[:])
            atts_t = cpool.tile([128, 4], f32)
            nc.sync.dma_start(atts_t[:], attb_s[:])
            attd_t = cpool.tile([128, 4], f32)
            nc.sync.dma_start(attd_t[:], attb_d[:])
            bias_t = cpool.tile([128, HC], f32)
            nc.sync.dma_start(bias_t[:], bias_bc[:])
            W1_t = cpool.tile([128, 256], f32)
            nc.sync.dma_start(W1_t[:], W1_d[:])
            b1_t = cpool.tile([128, 2], f32)
            nc.sync.dma_start(b1_t[:], b1_d[:])
            W2_t = cpool.tile([128, 2, HC], f32)
            nc.sync.dma_start(W2_t[:], W2_d.rearrange("(b p) m -> p b m", p=128))
            b2_t = cpool.tile([128, 1], f32)
            nc.sync.dma_start(b2_t[:], b2_d[:])
            W3_t = cpool.tile([128, 10], f32)
            nc.sync.dma_start(W3_t[:], W3_d[:])
            b3_t = cpool.tile([128, 1], f32)
            nc.sync.dma_start(b3_t[:], b3_d[:])
            eye_t = cpool.tile([128, 128], f32)
            nc.sync.dma_start(eye_t[:], eye_d[:])
            iota_t = cpool.tile([128, 128], f32)
            nc.sync.dma_start(iota_t[:], iota_d[:])
            ones_t = cpool.tile([128, 16], f32)
            nc.sync.dma_start(ones_t[:], ones_d[:])
            isrc_t = cpool.tile([128, TOTIDX // 16], i16)
            nc.sync.dma_start(isrc_t[:], idx_src[:])
            iad_t = cpool.tile([128, TOTIDX // 16], i16)
            nc.sync.dma_start(iad_t[:], idx_ad[:])
            dcol_t = cpool.tile([128, TOTC], f32)
            nc.sync.dma_start(dcol_t[:], dst_col[:])

            # WgT, Wgatt_s/d
            WgT_t = cpool.tile([128, 128], f32)
            Wgatt_t = cpool.tile([128, 8], f32)
            with tc.tile_pool(name="cpsum", bufs=1, space="PSUM") as cpsum:
                WgT_ps = cpsum.tile([128, 128], f32)
                nc.tensor.transpose(WgT_ps[:], Wg_t[:], eye_t[:])
                nc.vector.tensor_copy(WgT_t[:], WgT_ps[:])
                Wgatt_ps = cpsum.tile([128, 8], f32)
                nc.tensor.matmul(Wgatt_ps[:, 0:4], WgT_t[:], atts_t[:])
                nc.tensor.matmul(Wgatt_ps[:, 4:8], WgT_t[:], attd_t[:])
                nc.vector.tensor_copy(Wgatt_t[:], Wgatt_ps[:])

            # ================= stage A =================
            with (
                tc.tile_pool(name="sa_sb", bufs=2) as sa,
                tc.tile_pool(name="sa_ps", bufs=2, space="PSUM") as saps,
                tc.tile_pool(name="sa_ps1", bufs=2, space="PSUM") as saps1,
            ):
                QEND = {24: 0, 49: 1}
                QSR = [0, 3072, NPC]
                HFULL = [None, None]
                for g0 in range(0, TPC, 4):
                    gsz = min(4, TPC - g0)
                    gn = gsz * NT
                    xts = []
                    for u in range(gsz):
                        xt = sa.tile([128, NFP], f32, tag="xt", bufs=8)
                        nc.sync.dma_start(
                            xt[:], x_sl[(g0 + u) * NT:(g0 + u + 1) * NT, :])
                        xts.append(xt)
                    xT = sa.tile([128, 5, gn], f32, tag="xT")
                    for b in range(5):
                        for u in range(gsz):
                            tp = saps.tile([128, 128], f32, tag="tp", bufs=4)
                            nc.tensor.transpose(
                                tp[:], xts[u][:, b * 128:(b + 1) * 128], eye_t[:])
                            nc.scalar.activation(
                                xT[:, b, u * NT:(u + 1) * NT], tp[:], AF.Copy)
                    h1T_ps = saps.tile([128, gn], f32, tag="h1T")
                    for b in range(5):
                        nc.tensor.matmul(
                            h1T_ps[:], fb_t[:, b, :], xT[:, b, :],
                            start=(b == 0), stop=(b == 4))
                    h1T = sa.tile([128, gn], f32, tag="h1Ts")
                    nc.vector.tensor_copy(h1T[:], h1T_ps[:])
                    for u in range(gsz):
                        h_ps = saps1.tile([128, HC + 8], f32, tag="hps")
                        lhs = h1T[:, u * NT:(u + 1) * NT]
                        nc.tensor.matmul(h_ps[:, 0:HC], lhs, Wg_t[:])
                        nc.tensor.matmul(h_ps[:, HC:HC + 8], lhs, Wgatt_t[:])
                        hrow = sa.tile([128, 132], hdt, tag="hrow")
                        nc.vector.tensor_copy(hrow[:, 0:128], h_ps[:, 0:HC])
                        nc.vector.tensor_copy(
                            hrow[:, 128:132], h_ps[:, HC:HC + 4])
                        r0 = (g0 + u) * NT
                        nc.sync.dma_start(
                            Hext_loc[r0:r0 + NT, 0:132], hrow[:])
                        adr = sa.tile([128, ADR], hdt, tag="adr")
                        src_ap = bass.AP(
                            h_ps.tensor, h_ps.offset + (HC + 4),
                            [h_ps.ap[0], [0, ADR // 4], [1, 4]])
                        nc.vector.tensor_copy(
                            adr[:].rearrange("p (a b) -> p a b", a=ADR // 4, b=4), src_ap)
                        nc.sync.dma_start(adrep[r0:r0 + NT, :], adr[:])
                    if _PHASE >= 2 and (g0 + gsz) in QEND:
                        q = QEND[g0 + gsz]
                        hf_out = Hfull_a if q == 0 else Hfull_b
                        nc.gpsimd.collective_compute(
                            "AllGather",
                            mybir.AluOpType.bypass,
                            ins=[Hext_loc[QSR[q]:QSR[q + 1], :]],
                            outs=[hf_out[:]],
                            replica_groups=[core_ids],
                        )

            # ================= edge phase + MLP =================
            coffs = np.concatenate([[0], np.cumsum(cpt.sum(axis=1))]).astype(int)
            CPTA_MAX = int(cpt[:, 0].max())
            CPTB_MAX = int(cpt[:, 1].max())
            TOT_MAX = int((cpt[:, 0] + cpt[:, 1]).max())

            with (
                tc.tile_pool(name="eg_g", bufs=3) as egg,
                tc.tile_pool(name="eg_sb", bufs=2) as egs,
                tc.tile_pool(name="eg_acc", bufs=3, space="PSUM") as egacc,
                tc.tile_pool(name="eg_tp", bufs=2, space="PSUM") as egtp,
                tc.tile_pool(name="mlp_sb", bufs=2) as msb,
                tc.tile_pool(name="mlp_ps", bufs=1, space="PSUM") as mps,
            ):
                actT4 = None
                if _PHASE < 3:
                    dummy = msb.tile([16, 512], f32, tag="dummy")
                    nc.vector.memset(dummy[:], 0.125)
                    for g0 in range(0, TPC, 4):
                        gn = min(4, TPC - g0) * NT
                        nc.sync.dma_start(
                            outT[:, g0 * NT:g0 * NT + gn], dummy[0:10, 0:gn])
                for s in range(_PHASE >= 3 and TPC or 0):
                    cA, cB = int(cpt[s, 0]), int(cpt[s, 1])
                    tot = cA + cB
                    coff = int(coffs[s])

                    acc = egacc.tile([128, 132], f32, tag="acc")
                    nc.vector.memset(acc[:], 0.0)

                    ad = egg.tile([128, TOT_MAX, ADR], hdt, tag="ad")
                    nc.gpsimd.dma_gather(
                        ad[:, 0:tot, :], adrep[:],
                        iad_t[:, coff * 8:(coff + tot) * 8],
                        num_idxs=tot * NT, num_idxs_reg=tot * NT, elem_size=ADR,
                        single_packet=False)

                    halves = []
                    gA = egg.tile([128, CPTA_MAX, RDX], hdt, tag="gA")
                    nc.gpsimd.dma_gather(
                        gA[:, 0:cA, :], Hfull_a[:],
                        isrc_t[:, coff * 8:(coff + cA) * 8],
                        num_idxs=cA * NT, num_idxs_reg=cA * NT, elem_size=RDX,
                        single_packet=False)
                    halves.append((gA, 0, cA))
                    gB = egg.tile([128, CPTB_MAX, RDX], hdt, tag="gB")
                    nc.gpsimd.dma_gather(
                        gB[:, 0:cB, :], Hfull_b[:],
                        isrc_t[:, (coff + cA) * 8:(coff + tot) * 8],
                        num_idxs=cB * NT, num_idxs_reg=cB * NT, elem_size=RDX,
                        single_packet=False)
                    halves.append((gB, cA, cB))

                    ind = egs.tile([128, TOT_MAX, W_IND], hdt, tag="ind")
                    for (gt, c0, nh) in halves:
                        if nh == 0 or _KSUB < 2:
                            continue
                        # t = a_s + a_d  -> ex = exp(lrelu(t))
                        ex = egs.tile([128, TOT_MAX, 4], f32, tag="ex", bufs=2)
                        nc.vector.tensor_tensor(
                            ex[:, 0:nh, :], gt[:, 0:nh, 128:132],
                            ad[:, c0:c0 + nh, 0:4], OP.add)
                        nc.scalar.activation(
                            ex[:, 0:nh, :], ex[:, 0:nh, :], AF.Lrelu,
                            alpha=NEG_ATT)
                        nc.scalar.activation(
                            ex[:, 0:nh, :], ex[:, 0:nh, :], AF.Exp)
                        # msg *= ex (per head block)
                        if _BF16:
                            exs = egs.tile([128, TOT_MAX, 4], hdt, tag="exs",
                                           bufs=2)
                            nc.vector.tensor_copy(exs[:, 0:nh, :], ex[:, 0:nh, :])
                        else:
                            exs = ex
                        g4 = bass.AP(
                            gt.tensor, gt.offset,
                            [gt.ap[0], [RDX, nh], [32, 4], [1, 32]])
                        exb = bass.AP(
                            exs.tensor, exs.offset,
                            [exs.ap[0], [4, nh], [1, 4], [0, 32]])
                        nc.vector.tensor_tensor(g4, g4, exb, OP.mult)
                        # ex -> cols 128:132
                        nc.vector.tensor_copy(gt[:, 0:nh, 128:132], exs[:, 0:nh, :])
                        # indicator
                        iob = bass.AP(
                            iota_t.tensor, iota_t.offset,
                            [iota_t.ap[0], [0, nh], [1, W_IND]])
                        dcb = bass.AP(
                            dcol_t.tensor, dcol_t.offset + coff + c0,
                            [dcol_t.ap[0], [1, nh], [0, W_IND]])
                        nc.vector.tensor_tensor(
                            ind[:, c0:c0 + nh, :], iob, dcb, OP.is_equal)
                        for c in range(_KSUB >= 3 and nh or 0):
                            wo = int(woff[coff + c0 + c])
                            wl = int(wlen[coff + c0 + c])
                            nc.tensor.matmul(
                                acc[wo:wo + wl, :],
                                ind[:, c0 + c, 0:wl], gt[:, c, 0:132],
                                start=False, stop=(c0 + c == tot - 1),
                                skip_group_check=True)

                    # normalize + bias + ELU (node-major)
                    if _KSUB < 4:
                        gat0 = egs.tile([128, 128], f32, tag="gat")
                        if _KSUB >= 3:
                            nc.vector.tensor_copy(gat0[:], acc[:, 0:128])
                        else:
                            nc.vector.tensor_copy(gat0[:], gt[:, 0, 0:128])
                        sub = s % 4
                        if sub == 0:
                            gsz = min(4, TPC - s)
                            actT4 = msb.tile([128, 4 * NT], f32, tag="actT4")
                        nc.vector.tensor_copy(
                            actT4[:, sub * NT:(sub + 1) * NT], gat0[:])
                        if sub == gsz - 1:
                            nc.sync.dma_start(
                                outT[:, (s - sub) * NT:(s - sub) * NT + gsz * NT],
                                actT4[0:10, 0:gsz * NT])
                        continue
                    dinv = egs.tile([128, 4], f32, tag="dinv")
                    nc.vector.tensor_scalar(
                        dinv[:], acc[:, 128:132], 1e-12, None, OP.add)
                    nc.vector.reciprocal(dinv[:], dinv[:])
                    gat = egs.tile([128, 128], f32, tag="gat")
                    ga = bass.AP(gat.tensor, gat.offset,
                                 [gat.ap[0], [32, 4], [1, 32]])
                    aa = bass.AP(acc.tensor, acc.offset,
                                 [acc.ap[0], [32, 4], [1, 32]])
                    db = bass.AP(dinv.tensor, dinv.offset,
                                 [dinv.ap[0], [1, 4], [0, 32]])
                    nc.vector.tensor_tensor(ga, aa, db, OP.mult)
                    nc.vector.tensor_tensor(gat[:], gat[:], bias_t[:], OP.add)
                    # ELU = relu(x) - relu(1 - exp(x))
                    t1 = egs.tile([128, 128], f32, tag="t1")
                    nc.scalar.activation(t1[:], gat[:], AF.Exp)
                    nc.scalar.activation(t1[:], t1[:], AF.Relu, scale=-1.0, bias=1.0)
                    nc.scalar.activation(gat[:], gat[:], AF.Relu)
                    nc.vector.tensor_sub(gat[:], gat[:], t1[:])
                    # transpose -> actT4
                    sub = s % 4
                    if sub == 0:
                        gsz = min(4, TPC - s)
                        actT4 = msb.tile([128, 4 * NT], f32, tag="actT4")
                    tp = egtp.tile([128, 128], f32, tag="tp2")
                    nc.tensor.transpose(tp[:], gat[:], eye_t[:])
                    nc.vector.tensor_copy(actT4[:, sub * NT:(sub + 1) * NT], tp[:])

                    if _PHASE < 4 and sub == gsz - 1:
                        nc.sync.dma_start(
                            outT[:, (s - sub) * NT:(s - sub) * NT + gsz * NT],
                            actT4[0:10, 0:gsz * NT])
                    if _PHASE >= 4 and sub == gsz - 1:
                        g0 = s - sub
                        gn = gsz * NT
                        a1 = msb.tile([128, 2, 512], f32, tag="a1")
                        for j in range(2):
                            o1 = mps.tile([128, 512], f32, tag="o1")
                            nc.tensor.matmul(
                                o1[:, 0:gn], W1_t[:, j * 128:(j + 1) * 128],
                                actT4[:, 0:gn])
                            nc.scalar.activation(
                                a1[:, j, 0:gn], o1[:, 0:gn], AF.Lrelu,
                                alpha=NEG_MLP, bias=b1_t[:, j:j + 1])
                        o2 = mps.tile([128, 512], f32, tag="o2")
                        for j in range(2):
                            nc.tensor.matmul(
                                o2[:, 0:gn], W2_t[:, j, :], a1[:, j, 0:gn],
                                start=(j == 0), stop=(j == 1))
                        a2 = msb.tile([128, 512], f32, tag="a2")
                        nc.scalar.activation(
                            a2[:, 0:gn], o2[:, 0:gn], AF.Lrelu,
                            alpha=NEG_MLP, bias=b2_t[:])
                        o3 = mps.tile([16, 512], f32, tag="sm", name="o3_t")
                        nc.tensor.matmul(o3[0:10, 0:gn], W3_t[:], a2[:, 0:gn])
                        z = msb.tile([16, 512], f32, tag="z")
                        nc.scalar.activation(
                            z[0:10, 0:gn], o3[0:10, 0:gn], AF.Lrelu,
                            alpha=NEG_MLP, bias=b3_t[0:10, :])
                        nc.scalar.activation(z[0:10, 0:gn], z[0:10, 0:gn], AF.Exp)
                        ssum = mps.tile([16, 512], f32, tag="sm", name="ssum_t")[0:1, :]
                        nc.tensor.matmul(
                            ssum[:, 0:gn], ones_t[0:10, 0:1], z[0:10, 0:gn])
                        sinv = msb.tile([1, 512], f32, tag="sinv")
                        nc.vector.reciprocal(sinv[:, 0:gn], ssum[:, 0:gn])
                        sx = mps.tile([16, 512], f32, tag="sm", name="sx_t")
                        nc.tensor.matmul(
                            sx[0:10, 0:gn], ones_t[0:1, 0:10], sinv[:, 0:gn])
                        res = msb.tile([16, 512], f32, tag="res")
                        nc.vector.tensor_mul(
                            res[0:10, 0:gn], z[0:10, 0:gn], sx[0:10, 0:gn])
                        nc.sync.dma_start(
                            outT[:, g0 * NT:g0 * NT + gn], res[0:10, 0:gn])

    nc.compile()
    return nc


def _inputs_per_core(inputs, src_w, ad_w, dst_col, meta):
    x = np.asarray(inputs["x"], dtype=np.float32)
    fb = np.asarray(inputs["fb"], dtype=np.float32)
    Wg = np.asarray(inputs["Wg"], dtype=np.float32)
    bias_g = np.asarray(inputs["bias_g"], dtype=np.float32)
    att_src = np.asarray(inputs["att_src"], dtype=np.float32)
    att_dst = np.asarray(inputs["att_dst"], dtype=np.float32)
    W1 = np.asarray(inputs["W1"], dtype=np.float32)
    b1 = np.asarray(inputs["b1"], dtype=np.float32)
    W2 = np.asarray(inputs["W2"], dtype=np.float32)
    b2 = np.asarray(inputs["b2"], dtype=np.float32)
    W3 = np.asarray(inputs["W3"], dtype=np.float32)
    b3 = np.asarray(inputs["b3"], dtype=np.float32)

    x_pad = np.zeros((NPAD, NFP), dtype=np.float32)
    x_pad[:N, :NF] = x
    fb_pad = np.zeros((NFP, NMEL), dtype=np.float32)
    fb_pad[:NF] = fb

    att_blk_s = np.zeros((HC, 4), dtype=np.float32)
    att_blk_d = np.zeros((HC, 4), dtype=np.float32)
    for h in range(H):
        att_blk_s[h * C:(h + 1) * C, h] = att_src[h]
        att_blk_d[h * C:(h + 1) * C, h] = att_dst[h]

    b1p = np.zeros((128, 2), dtype=np.float32)
    b1p[:, 0] = b1[:128]
    b1p[:, 1] = b1[128:]
    b2p = b2.reshape(128, 1).astype(np.float32)
    b3p = np.zeros((128, 1), dtype=np.float32)
    b3p[:10, 0] = b3

    common = {
        "fb_p": fb_pad, "Wg": Wg, "attb_s": att_blk_s, "attb_d": att_blk_d,
        "bias_bc": np.tile(bias_g[None, :], (128, 1)).astype(np.float32),
        "W1": W1, "b1": b1p, "W2": W2, "b2": b2p, "W3": W3, "b3": b3p,
        "eye": np.eye(128, dtype=np.float32),
        "iota": np.tile(np.arange(128, dtype=np.float32)[None, :], (128, 1)),
        "ones": np.ones((128, 16), dtype=np.float32),
    }
    maps = []
    for k in range(NCORES):
        m = dict(common)
        m["x_sl"] = np.ascontiguousarray(x_pad[k * NPC:(k + 1) * NPC])
        m["idx_src"] = src_w[k]
        m["idx_ad"] = ad_w[k]
        m["dst_col"] = dst_col[k]
        maps.append(m)
    return maps


def kernel(**inputs):
    from concourse.bass_utils import run_bass_kernel_spmd

    src_w, ad_w, dst_col, meta = _prep(inputs["edge_index"])
    key = ("nc", meta["TOTC"], tuple(meta["cpt"].reshape(-1)),
           tuple(meta["woff"]))
    if key not in _CACHE:
        _CACHE.clear()
        _CACHE[key] = _build(meta)
    nc = _CACHE[key]
    maps = _inputs_per_core(inputs, src_w, ad_w, dst_col, meta)
    res = run_bass_kernel_spmd(nc, maps, core_ids=list(range(NCORES)))
    out = np.zeros((NPAD, 10), dtype=np.float32)
    for k in range(NCORES):
        out[k * NPC:(k + 1) * NPC] = res.results[k]["outT"].T
    return out[:N]

